# revision 5
# baseline (speedup 1.0000x reference)
"""CirLinear Trainium2 kernel v3: y = x @ build_weight(W, alphas, gumbels)^T + bias.

Strategy (8 NeuronCores, no collectives), 2x4 grid:
 - core c = tshard*4 + oshard: tokens [8192*tshard, +8192), out rows
   [512*oshard, +512)
 - HOST-TILED contiguous DMA layouts: x, W and the output are pre/post
   shuffled on the host so every big device DMA is a flat contiguous
   block (no strided descriptors)
 - DMA ring split: token tiles on the SP HWDGE ring, weight-build
   traffic (ws load, w_loc roundtrip, xbar transposes) on the ACT HWDGE
   ring, output stores on the GPSIMD SWDGE path -- the three streams
   don't serialize behind each other
 - circulant weight build in bf16: pair-tree diagonal reduce
   (tensor_tensor 2x mode), doubled-pad copies on ACT + GPSIMD,
   flat tensor_scalar rescale, tensor_tensor expansion
 - built chunk (256 rows) roundtrips DRAM for the transpose into wT
   (xbar transpose DMA), feeding bf16 matmuls
 - matmul phase software-pipelined: o-rows {0,1} (chunk 0) run DELTA
   token-groups ahead of o-rows {2,3} (chunk 1)
 - fp32 PSUM accumulation over 16 K-chunks, bias added on ACT, bf16
   output tiles; host assembles and casts to f32
"""
import sys

sys.path.insert(0, '/opt/trn_rl_repo')

import numpy as np

import concourse.bass as bass
from concourse import bacc
import concourse.mybir as mybir
from concourse.tile import TileContext
from concourse.bass_utils import run_bass_kernel_spmd

N_CORES = 8
T_SHARDS, O_SHARDS = 2, 4
BATCH, TOKENS, IN_F, OUT_F = 16, 1024, 2048, 2048
TOK_TOTAL = BATCH * TOKENS            # 16384
TOK = TOK_TOTAL // T_SHARDS           # 8192 tokens per core
ROWS = OUT_F // O_SHARDS              # 512 out-features per core
N_CH = ROWS // 256                    # 2 build chunks of 256 rows
SCALES = [2, 4, 8, 16, 32, 64]
N_IC = IN_F // 128                    # 16 contraction chunks
N_TG = TOK // 512                     # 16 token groups of 512
N_OS = ROWS // 128                    # 4 output-row subtiles
DELTA = 4                             # o23 pipeline delay in token-groups

bf16 = mybir.dt.bfloat16
f32 = mybir.dt.float32

_CACHE = {}


def _build_nc():
    nc = bacc.Bacc("TRN2", target_bir_lowering=False, debug=False, num_devices=N_CORES)
    # host-tiled inputs: every per-tg / per-chunk DMA is contiguous
    xt_d = nc.dram_tensor("xt_d", [N_TG * 128, N_IC * 512], bf16, kind="ExternalInput")
    ws_t = nc.dram_tensor("ws_t", [N_CH * 128, 4096], bf16, kind="ExternalInput")
    # coef: host-precomputed softmax mix + bias, broadcast per partition:
    # cols 0..6 = a (softmax), 7..13 = a/b per scale, 14..17 = bias per o-subtile
    coef = nc.dram_tensor("coef", [128, 18], f32, kind="ExternalInput")
    out_t = nc.dram_tensor("out_t", [N_TG * 128, 2048], bf16, kind="ExternalOutput")

    w_loc = nc.dram_tensor("w_loc", [ROWS, IN_F], bf16)

    with TileContext(nc) as tc:
        coef_sb = nc.alloc_sbuf_tensor("coef_sb", [128, 18], f32).ap()
        nc.scalar.dma_start(out=coef_sb, in_=coef.ap())

        # ---------- circulant weight build: 2 chunks of 256 rows ----------
        # chunk partition = (q4, p32) : 128 ; free = (r64, s64) : 64*64 = 4096
        wb2 = [nc.alloc_sbuf_tensor(f"wb{i}", [128, 4096], bf16).ap() for i in range(2)]
        acc = nc.alloc_sbuf_tensor("acc", [128, 4096], bf16).ap()
        wbpad2 = [nc.alloc_sbuf_tensor(f"wbpad{i}", [128, 8192], bf16).ap() for i in range(2)]
        etree = nc.alloc_sbuf_tensor("etree", [128, 4096], bf16).ap()
        dpad = nc.alloc_sbuf_tensor("dpad", [128, 4096], bf16).ap()
        wloc_4d = w_loc.ap().rearrange("(q r) (p s) -> q p r s", r=64, s=64)

        wT = nc.alloc_sbuf_tensor("wT", [128, N_IC * 512], bf16).ap()

        def sb(t, off, dims):
            return bass.AP(tensor=t.tensor, offset=off, ap=[list(t.ap[0])] + dims)

        for ch in range(N_CH):
            # contiguous 1 MB chunk load on the ACT ring (double-buffered by chunk)
            wb = wb2[ch % 2]
            nc.scalar.dma_start(out=wb,
                                in_=bass.AP(tensor=ws_t, offset=ch * 128 * 4096,
                                            ap=[[4096, 128], [1, 4096]]))
            nc.vector.tensor_scalar_mul(acc, wb, coef_sb[:, 0:1])
            for idx, b in enumerate(SCALES, start=1):
                nv = 64 // b
                nu = nv
                # doubled pad: wbpad[r, v, half*b + t] = wb[r, v*b + t].
                # Ping-pong buffer by scale parity; one engine writes BOTH
                # halves (ACT for 4 scales, GPSIMD for 2) so consecutive
                # scales' pads never WAR-serialize against the tree reads.
                wbpad = wbpad2[idx % 2]
                src = sb(wb, 0, [[64, 64], [b, nv], [1, b]])
                if b in (2, 8):
                    pad_engine = nc.gpsimd.tensor_copy
                elif b == 64:
                    pad_engine = nc.vector.tensor_copy
                else:
                    pad_engine = nc.scalar.copy
                for half in range(2):
                    pad_engine(out=sb(wbpad, half * b, [[128, 64], [2 * b, nv], [1, b]]),
                               in_=src)
                # pair-tree diagonal reduce (all tensor_tensor, bf16 2x mode),
                # each instruction <= 3 free dims for the BIR verifier.
                # level 0: E[u,v,m,t] = D[2m, t+2m] + D[2m+1, t+2m+1]
                add = mybir.AluOpType.add
                m2 = b // 2
                if b == 2:
                    # level 0 IS the final level: emit twice into dpad
                    for half in range(2):
                        nc.vector.tensor_tensor(
                            out=sb(dpad, half * b, [[128, nv], [2 * b, nv], [1, b]]),
                            in0=sb(wbpad, 0, [[b * 128, nu], [2 * b, nv], [1, b]]),
                            in1=sb(wbpad, 129, [[b * 128, nu], [2 * b, nv], [1, b]]),
                            op=add)
                else:
                    if m2 <= nu:
                        for m in range(m2):
                            nc.vector.tensor_tensor(
                                out=sb(etree, m * b, [[nv * m2 * b, nu], [m2 * b, nv], [1, b]]),
                                in0=sb(wbpad, m * 258, [[b * 128, nu], [2 * b, nv], [1, b]]),
                                in1=sb(wbpad, m * 258 + 129, [[b * 128, nu], [2 * b, nv], [1, b]]),
                                op=add)
                    else:
                        for u in range(nu):
                            nc.vector.tensor_tensor(
                                out=sb(etree, u * nv * m2 * b, [[m2 * b, nv], [b, m2], [1, b]]),
                                in0=sb(wbpad, u * b * 128, [[2 * b, nv], [258, m2], [1, b]]),
                                in1=sb(wbpad, u * b * 128 + 129, [[2 * b, nv], [258, m2], [1, b]]),
                                op=add)
                    off = 0
                    size = nu * nv * m2 * b
                    # mid levels halve m until the input pair level (m2 == 2)
                    while m2 > 2:
                        m2h = m2 // 2
                        prev_off, prev_m2 = off, m2
                        off += size
                        if m2h <= nu:
                            for j in range(m2h):
                                nc.vector.tensor_tensor(
                                    out=sb(etree, off + j * b,
                                           [[nv * m2h * b, nu], [m2h * b, nv], [1, b]]),
                                    in0=sb(etree, prev_off + 2 * j * b,
                                           [[nv * prev_m2 * b, nu], [prev_m2 * b, nv], [1, b]]),
                                    in1=sb(etree, prev_off + (2 * j + 1) * b,
                                           [[nv * prev_m2 * b, nu], [prev_m2 * b, nv], [1, b]]),
                                    op=add)
                        else:
                            for u in range(nu):
                                nc.vector.tensor_tensor(
                                    out=sb(etree, off + u * nv * m2h * b,
                                           [[m2h * b, nv], [b, m2h], [1, b]]),
                                    in0=sb(etree, prev_off + u * nv * prev_m2 * b,
                                           [[prev_m2 * b, nv], [2 * b, m2h], [1, b]]),
                                    in1=sb(etree, prev_off + u * nv * prev_m2 * b + b,
                                           [[prev_m2 * b, nv], [2 * b, m2h], [1, b]]),
                                    op=add)
                        m2 = m2h
                        size = nu * nv * m2 * b
                    # final level (input m2 == 2): double-write into dpad
                    for half in range(2):
                        nc.vector.tensor_tensor(
                            out=sb(dpad, half * b, [[128, nv], [2 * b, nv], [1, b]]),
                            in0=sb(etree, off, [[nv * 2 * b, nu], [2 * b, nv], [1, b]]),
                            in1=sb(etree, off + b, [[nv * 2 * b, nu], [2 * b, nv], [1, b]]),
                            op=add)
                # scale the whole (contiguous) dpad by a_idx / b in one flat op
                dflat = sb(dpad, 0, [[1, nv * nv * 2 * b]])
                nc.vector.tensor_scalar_mul(dflat, dflat, coef_sb[:, 7 + idx:8 + idx])
                # expansion: acc[u,v,i,s] += dpad[u, v, b + s - i]
                if nv <= b:
                    for u in range(nu):
                        aout = sb(acc, u * b * 64, [[b, nv], [64, b], [1, b]])
                        din = sb(dpad, u * 128 + b, [[2 * b, nv], [-1, b], [1, b]])
                        nc.vector.tensor_tensor(out=aout, in0=aout, in1=din, op=add)
                else:
                    for r in range(b):
                        aout = sb(acc, r * 64, [[b * 64, nv], [b, nv], [1, b]])
                        din = sb(dpad, b - r, [[128, nv], [2 * b, nv], [1, b]])
                        nc.vector.tensor_tensor(out=aout, in0=aout, in1=din, op=add)
            # scatter chunk -> w_loc rows [256ch, 256ch+256) on the ACT ring
            for q in range(4):
                nc.scalar.dma_start(out=wloc_4d[ch * 4 + q], in_=acc[q * 32:(q + 1) * 32, :])
            # transposed reload of this chunk's columns into wT (ACT ring)
            for ic in range(N_IC):
                nc.scalar.dma_start(out=wT[:, ic * 512 + ch * 256:ic * 512 + (ch + 1) * 256],
                                    in_=w_loc.ap()[ch * 256:(ch + 1) * 256, ic * 128:(ic + 1) * 128],
                                    transpose=True)

        # ---------- main matmul: o01 pass runs DELTA token-groups ahead of o23 ----------
        with (
            tc.tile_pool(name="xt", bufs=DELTA + 2) as xt_pool,
            tc.tile_pool(name="psum", bufs=8, space="PSUM") as psum_pool,
            tc.tile_pool(name="osb", bufs=2) as osb_pool,
        ):
            xts = [None] * N_TG

            def half_step(tg, os_pair, ot_name, half):
                rhs_all = xts[tg]
                psums = [psum_pool.tile([128, 512], f32, name=f"ps{o}", tag="ps")
                         for o in os_pair]
                for ic in range(N_IC):
                    rhs = rhs_all[:, ic * 512:(ic + 1) * 512]
                    for j, o in enumerate(os_pair):
                        nc.tensor.matmul(psums[j][:], wT[:, ic * 512 + o * 128:ic * 512 + (o + 1) * 128],
                                         rhs, start=(ic == 0), stop=(ic == N_IC - 1))
                ot = osb_pool.tile([128, 1024], bf16, name=ot_name, tag=ot_name)
                for j, o in enumerate(os_pair):
                    nc.scalar.activation(out=ot[:, j * 512:(j + 1) * 512], in_=psums[j][:],
                                         func=mybir.ActivationFunctionType.Identity,
                                         bias=coef_sb[:, 14 + o:15 + o], scale=1.0)
                # contiguous store on the SWDGE path
                nc.gpsimd.dma_start(
                    out=bass.AP(tensor=out_t, offset=tg * 128 * 2048 + half * 1024,
                                ap=[[2048, 128], [1, 1024]]),
                    in_=ot[:])

            for k in range(N_TG + DELTA):
                if k < N_TG:
                    xts[k] = xt_pool.tile([128, N_IC * 512], bf16, name="xt")
                    nc.sync.dma_start(
                        out=xts[k][:],
                        in_=bass.AP(tensor=xt_d, offset=k * 128 * 8192,
                                    ap=[[8192, 128], [1, 8192]]))
                    half_step(k, (0, 1), "ot01", 0)
                if k >= DELTA:
                    half_step(k - DELTA, (2, 3), "ot23", 1)

    nc.compile()
    return nc


def make_in_maps(x, weight, bias, alphas, gumbels):
    import ml_dtypes
    x2 = np.asarray(x, np.float32).reshape(TOK_TOTAL, IN_F)
    xTh = np.ascontiguousarray(x2.T).astype(ml_dtypes.bfloat16)   # [2048, 16384]
    xslices = []
    for t in range(T_SHARDS):
        a = xTh[:, t * TOK:(t + 1) * TOK].reshape(N_IC, 128, N_TG, 512)
        a = np.ascontiguousarray(a.transpose(2, 1, 0, 3)).reshape(N_TG * 128, N_IC * 512)
        xslices.append(a)
    weight = np.asarray(weight, np.float32)
    bias = np.asarray(bias, np.float32)
    wslices = []
    for o in range(O_SHARDS):
        w_o = weight[o * ROWS:(o + 1) * ROWS].astype(ml_dtypes.bfloat16)
        b4 = w_o.reshape(N_CH, 4, 64, 32, 64).transpose(0, 1, 3, 2, 4)
        wslices.append(np.ascontiguousarray(b4).reshape(N_CH * 128, 4096))
    # host-side softmax + per-scale rescale; bias in the per-partition layout
    logits = np.asarray(alphas, np.float64) + np.asarray(gumbels, np.float64)
    e = np.exp(logits - logits.max())
    a = (e / e.sum()).astype(np.float32)                       # [7]
    cslices = []
    for o in range(O_SHARDS):
        coef = np.zeros((128, 18), np.float32)
        coef[:, 0:7] = a
        coef[:, 7:14] = a / np.array([1] + SCALES, np.float32)
        b_o = bias[o * ROWS:(o + 1) * ROWS].reshape(N_OS, 128)
        coef[:, 14:18] = b_o.T
        cslices.append(coef)
    in_maps = []
    for c in range(N_CORES):
        t, o = divmod(c, O_SHARDS)
        in_maps.append({"xt_d": xslices[t], "ws_t": wslices[o], "coef": cslices[o]})
    return in_maps


def _untile_out(out_t):
    """[N_TG*128, 2048] bf16 tile layout -> [TOK, ROWS] f32 block."""
    o5 = np.asarray(out_t).reshape(N_TG, 128, 2, 2, 512)       # tg, p, h, j, c
    o5 = o5.transpose(0, 4, 2, 3, 1)                           # tg, c, h, j, p
    return np.ascontiguousarray(o5).reshape(TOK, ROWS).astype(np.float32)


def kernel(x, weight, bias, alphas, gumbels):
    if "nc" not in _CACHE:
        _CACHE["nc"] = _build_nc()
    nc = _CACHE["nc"]
    in_maps = make_in_maps(x, weight, bias, alphas, gumbels)
    res = run_bass_kernel_spmd(nc, in_maps, core_ids=list(range(N_CORES)))
    y = np.empty((TOK_TOTAL, OUT_F), np.float32)
    for c in range(N_CORES):
        t, o = divmod(c, O_SHARDS)
        y[t * TOK:(t + 1) * TOK, o * ROWS:(o + 1) * ROWS] = _untile_out(res.results[c]["out_t"])
    return y.reshape(BATCH, TOKENS, OUT_F)
